# revision 9
# baseline (speedup 1.0000x reference)
"""Trainium2 one-launch kernel for nn_Net_68994354643186 (3-layer
TransformerConv GNN).

Everything runs on the 8 NeuronCores in a SINGLE SPMD launch:
  - Nodes are partitioned into 8 contiguous shards of 6250 (edge-cut),
    and each core's shard is re-ordered by in-degree so the slot-padded
    edge tables are tight per 128-node tile (per-tile slot count K_t is
    specialized into the program at build time).
  - Per layer, each core computes the fused q|k|v|s projection GEMM for
    its shard (bf16, bias folded via ones-row, 1/sqrt(D) folded into Wq),
    writes a packed [k|v] bf16 row table, AllGathers it so every core
    holds the full 50000-row k|v table, then runs the per-edge attention
    for its own destination nodes: indirect-DMA row gathers (128 rows
    per instruction, one per dest-node partition), masked exp-softmax
    without max-subtraction (scores clamped at 60), strided slot-sum
    reductions, the root/skip add and LeakyReLU.  Layer outputs stay
    on-device (bf16, with a ones column for the next bias) and are
    re-loaded transposed via DMA-transpose as the next layer's GEMM
    input.
  - The host only builds the degree-sorted slot tables, uploads
    x^T / weights / packed slot indices+mask, and un-permutes the
    concatenated output shards.

The program is compiled and executed once for warm-up (AOT) before the
timed launch, so the timed launch is pure transfer + execute.

Self-contained: hardcodes all shapes; no sibling imports.
"""

import sys

sys.path.insert(0, "/opt/trn_rl_repo")

import numpy as np

try:
    import jax

    jax.config.update("jax_compilation_cache_dir", "/tmp/jax_cc_cache")
    jax.config.update("jax_persistent_cache_min_entry_size_bytes", 0)
    jax.config.update("jax_persistent_cache_min_compile_time_secs", 0.0)
except Exception:
    pass

N_NODES = 50000
N_EDGES = 800000
N_CORES = 8
SHARD = N_NODES // N_CORES  # 6250
LEAKY_ALPHA = 0.1
P = 128
NT = (SHARD + P - 1) // P  # 49 tiles; last has 106 real rows
SHARD_PAD = NT * P         # 6272

_LAYERS = [
    # (cin, heads, head_dim)
    (130, 4, 50),
    (200, 4, 25),
    (100, 4, 10),
]

_STATE = {}


def _build_program(KT, BANDS, BATCHES):
    """KT: per-tile slot counts; BANDS: [(t0,t1,Kb)]; BATCHES: [(t0,G,b)]."""
    import concourse.bass as bass
    import concourse.bacc as bacc
    import concourse.mybir as mybir
    import concourse.tile as tile

    fdt = mybir.dt.float32
    bdt = mybir.dt.bfloat16
    KMAX = max(KT)

    nc = bacc.Bacc("TRN2", num_devices=N_CORES)
    xt1_in = nc.dram_tensor("xt1", [131, SHARD], bdt, kind="ExternalInput")
    w_ins = []
    for li, (cin, H, D) in enumerate(_LAYERS):
        kf = 2 if cin + 1 > 128 else 1
        w_ins.append(
            nc.dram_tensor(f"w{li+1}", [128, kf, 4 * H * D], bdt, kind="ExternalInput")
        )
    idx_ins, mask_ins = [], []
    for b, (t0, t1, Kb) in enumerate(BANDS):
        rows = (t1 - t0) * P
        idx_ins.append(
            nc.dram_tensor(f"idx{b}", [rows, Kb], mybir.dt.int32, kind="ExternalInput")
        )
        mask_ins.append(
            nc.dram_tensor(f"mask{b}", [rows, Kb], bdt, kind="ExternalInput")
        )
    band_of = []
    for t in range(NT):
        for b, (t0, t1, Kb) in enumerate(BANDS):
            if t0 <= t < t1:
                band_of.append(b)
                break
    y_out = nc.dram_tensor("y", [SHARD, 40], bdt, kind="ExternalOutput")

    with tile.TileContext(nc) as tc:
        with (
            tc.tile_pool(name="wpool", bufs=1) as wpool,
            tc.tile_pool(name="sb", bufs=2) as sb,
            tc.tile_pool(name="gp", bufs=2) as gp,
            tc.tile_pool(name="res", bufs=1) as res,
            tc.tile_pool(name="psum", bufs=4, space="PSUM") as pspool,
            tc.tile_pool(name="dram", bufs=1, space="DRAM") as dram,
        ):
            wts = []
            for li, (cin, H, D) in enumerate(_LAYERS):
                kf = 2 if cin + 1 > 128 else 1
                wt = wpool.tile([128, kf, 4 * H * D], bdt, tag=f"w{li}")
                nc.sync.dma_start(out=wt[:], in_=w_ins[li][:])
                wts.append(wt)

            h_prev = None  # DRAM [SHARD_PAD, F_prev + 1] bf16 (ones col last)
            for li, (cin, H, D) in enumerate(_LAYERS):
                F = H * D
                FQ = F // 2  # f32 words holding the bf16 q row
                ELEM = 2 * F
                cr = cin + 1
                kf = 2 if cr > 128 else 1
                m4 = 4 * F
                wt = wts[li]

                kv_local = dram.tile([SHARD, ELEM], bdt, tag=f"kvl{li}")
                kv_full = dram.tile([N_NODES, ELEM], bdt, tag=f"kvf{li}")
                qres = res.tile([P, NT * F], bdt, tag="qres")
                sres = res.tile([P, NT * F], fdt, tag="sres")
                h_tab = None
                if li < 2:
                    h_tab = dram.tile([SHARD_PAD, F + 1], bdt, tag=f"ht{li}")

                # -------- pass A: projections for own shard --------
                NCH = []
                c0 = 0
                while c0 < m4:
                    cn = min(400, m4 - c0)
                    NCH.append((c0, cn))
                    c0 += cn
                for t in range(NT):
                    m0 = t * P
                    m = min(P, SHARD - m0)
                    xt_t = sb.tile([128, kf, P], bdt, tag="xt")
                    nc.vector.memset(xt_t[:], 0)
                    if li == 0:
                        nc.sync.dma_start(
                            out=xt_t[:128, 0, :m], in_=xt1_in[0:128, m0 : m0 + m]
                        )
                        nc.sync.dma_start(
                            out=xt_t[0:3, 1, :m], in_=xt1_in[128:131, m0 : m0 + m]
                        )
                    else:
                        # h_prev has a ones column at index fp: the transpose
                        # loads features AND the bias ones-row together.
                        fp = _LAYERS[li - 1][1] * _LAYERS[li - 1][2]
                        r1 = min(fp + 1, 128)
                        nc.sync.dma_start_transpose(
                            out=xt_t[:r1, 0, :], in_=h_prev[m0 : m0 + P, 0:r1]
                        )
                        if fp + 1 > 128:
                            nc.sync.dma_start_transpose(
                                out=xt_t[: fp + 1 - 128, 1, :],
                                in_=h_prev[m0 : m0 + P, 128 : fp + 1],
                            )
                    kvb = sb.tile([P, ELEM], bdt, tag="kvb")
                    # W columns are ordered q|s|k|v so k|v is one contiguous copy
                    for (c0, cn) in NCH:
                        ps = pspool.tile([P, 400], fdt, tag="ps")
                        for ki in range(kf):
                            nc.tensor.matmul(
                                ps[:m, :cn],
                                lhsT=xt_t[:, ki, :m],
                                rhs=wt[:, ki, c0 : c0 + cn],
                                start=(ki == 0),
                                stop=(ki == kf - 1),
                            )
                        for dst_ap, soff, w_ in (
                            (qres[:m, t * F : (t + 1) * F], 0, F),
                            (sres[:m, t * F : (t + 1) * F], F, F),
                            (kvb[:m, :], 2 * F, 2 * F),
                        ):
                            lo = max(soff, c0)
                            hi = min(soff + w_, c0 + cn)
                            if lo < hi:
                                nc.vector.tensor_copy(
                                    out=dst_ap[:, lo - soff : hi - soff],
                                    in_=ps[:m, lo - c0 : hi - c0],
                                )
                    nc.sync.dma_start(out=kv_local[m0 : m0 + m, :], in_=kvb[:m, :])

                # -------- AllGather the k|v table --------
                nc.gpsimd.collective_compute(
                    "AllGather",
                    mybir.AluOpType.bypass,
                    replica_groups=[list(range(N_CORES))],
                    ins=[kv_local[:]],
                    outs=[kv_full[:]],
                )

                # -------- pass B: attention, batched over G tiles --------
                for (bt0, G, b) in BATCHES:
                    Kb = BANDS[b][2]
                    GK = G * Kb
                    m0 = bt0 * P
                    mlast = min(P, SHARD - (bt0 + G - 1) * P)
                    full = GK if mlast == P else (G - 1) * Kb  # cols with all 128 rows valid
                    r0b = (bt0 - BANDS[b][0]) * P
                    it = sb.tile([P, GK], mybir.dt.int32, tag="it")
                    nc.sync.dma_start(
                        out=it[:].rearrange("p (g k) -> p g k", g=G),
                        in_=idx_ins[b][r0b : r0b + G * P, :].rearrange(
                            "(g p) k -> p g k", g=G
                        ),
                    )
                    mt = sb.tile([P, GK], bdt, tag="mt")
                    nc.sync.dma_start(
                        out=mt[:].rearrange("p (g k) -> p g k", g=G),
                        in_=mask_ins[b][r0b : r0b + G * P, :].rearrange(
                            "(g p) k -> p g k", g=G
                        ),
                    )
                    qt = qres[:, bt0 * F : (bt0 + G) * F]
                    st = sres[:, bt0 * F : (bt0 + G) * F]

                    g2 = gp.tile([P, GK, ELEM], bdt, tag="g")
                    for c in range(GK):
                        nc.gpsimd.indirect_dma_start(
                            out=g2[:, c, :],
                            out_offset=None,
                            in_=kv_full[:],
                            in_offset=bass.IndirectOffsetOnAxis(
                                ap=it[:, c : c + 1], axis=0
                            ),
                        )
                    # scores: in-place q*k product over the k half, then reduce
                    nc.vector.tensor_tensor(
                        out=g2[:, :, 0:F].rearrange("p (g k) f -> p g k f", g=G),
                        in0=g2[:, :, 0:F].rearrange("p (g k) f -> p g k f", g=G),
                        in1=qt.rearrange("p (g f) -> p g f", g=G)
                        .unsqueeze(2)
                        .broadcast_to([P, G, Kb, F]),
                        op=mybir.AluOpType.mult,
                    )
                    scores = sb.tile([P, GK * H], fdt, tag="scores")
                    nc.vector.tensor_reduce(
                        out=scores[:],
                        in_=g2[:, :, 0:F].rearrange("p c (h d) -> p c h d", h=H),
                        axis=mybir.AxisListType.X,
                        op=mybir.AluOpType.add,
                    )
                    sm = sb.tile([P, GK * H], fdt, tag="sm")
                    nc.vector.scalar_tensor_tensor(
                        out=sm[:].rearrange("p (c h) -> p c h", h=H),
                        in0=scores[:].rearrange("p (c h) -> p c h", h=H),
                        scalar=60.0,
                        in1=mt[:].to_broadcast([P, GK, H]),
                        op0=mybir.AluOpType.min,
                        op1=mybir.AluOpType.add,
                    )
                    es = sb.tile([P, GK * H], bdt, tag="es")
                    nc.scalar.activation(
                        out=es[:], in_=sm[:], func=mybir.ActivationFunctionType.Exp
                    )
                    dn = sb.tile([P, G * H], fdt, tag="dn")
                    nc.vector.tensor_reduce(
                        out=dn[:],
                        in_=es[:].rearrange("p (g k h) -> p g h k", g=G, k=Kb),
                        axis=mybir.AxisListType.X,
                        op=mybir.AluOpType.add,
                    )
                    # weighted v in place over the v half
                    nc.vector.tensor_tensor(
                        out=g2[:, :, F:ELEM].rearrange("p c (h d) -> p c h d", h=H),
                        in0=g2[:, :, F:ELEM].rearrange("p c (h d) -> p c h d", h=H),
                        in1=es[:]
                        .rearrange("p (c h) -> p c h", h=H)
                        .unsqueeze(3)
                        .broadcast_to([P, GK, H, D]),
                        op=mybir.AluOpType.mult,
                    )
                    osum = sb.tile([P, G * F], fdt, tag="osum")
                    nc.vector.tensor_reduce(
                        out=osum[:],
                        in_=g2[:, :, F:ELEM].rearrange("p (g k) f -> p g f k", g=G),
                        axis=mybir.AxisListType.X,
                        op=mybir.AluOpType.add,
                    )
                    rec = sb.tile([P, G * H], fdt, tag="rec")
                    nc.vector.reciprocal(out=rec[:], in_=dn[:])
                    hsb = sb.tile([P, G * F], fdt, tag="hsb")
                    nc.vector.tensor_tensor(
                        out=hsb[:].rearrange("p (g h d) -> p g h d", g=G, h=H),
                        in0=osum[:].rearrange("p (g h d) -> p g h d", g=G, h=H),
                        in1=rec[:]
                        .rearrange("p (g h) -> p g h", g=G)
                        .unsqueeze(3)
                        .broadcast_to([P, G, H, D]),
                        op=mybir.AluOpType.mult,
                    )
                    nc.vector.tensor_tensor(
                        out=hsb[:], in0=hsb[:], in1=st, op=mybir.AluOpType.add
                    )
                    if li < 2:
                        hb = sb.tile([P, G * (F + 1)], bdt, tag="hb")
                        nc.vector.scalar_tensor_tensor(
                            out=hb[:].rearrange("p (g f) -> p g f", g=G)[:, :, 0:F],
                            in0=hsb[:].rearrange("p (g f) -> p g f", g=G),
                            scalar=LEAKY_ALPHA,
                            in1=hsb[:].rearrange("p (g f) -> p g f", g=G),
                            op0=mybir.AluOpType.mult,
                            op1=mybir.AluOpType.max,
                        )
                        nc.vector.memset(
                            hb[:].rearrange("p (g f) -> p g f", g=G)[:, :, F : F + 1],
                            1.0,
                        )
                        if mlast == P:
                            nc.sync.dma_start(
                                out=h_tab[m0 : m0 + G * P, :].rearrange(
                                    "(g p) f -> p g f", g=G
                                ),
                                in_=hb[:].rearrange("p (g f) -> p g f", g=G),
                            )
                        else:
                            # ragged tile is always its own G=1 batch
                            nc.sync.dma_start(
                                out=h_tab[m0 : m0 + mlast, :], in_=hb[:mlast, :]
                            )
                    else:
                        negm = sb.tile([P, G], fdt, tag="negm")
                        nc.vector.tensor_reduce(
                            out=negm[:],
                            in_=hsb[:].rearrange("p (g f) -> p g f", g=G),
                            axis=mybir.AxisListType.X,
                            op=mybir.AluOpType.max,
                            negate=True,
                        )
                        z = sb.tile([P, G * F], fdt, tag="z")
                        nc.vector.tensor_tensor(
                            out=z[:].rearrange("p (g f) -> p g f", g=G),
                            in0=hsb[:].rearrange("p (g f) -> p g f", g=G),
                            in1=negm[:].unsqueeze(2).broadcast_to([P, G, F]),
                            op=mybir.AluOpType.add,
                        )
                        ez = sb.tile([P, G * F], fdt, tag="ez")
                        nc.scalar.activation(
                            out=ez[:], in_=z[:], func=mybir.ActivationFunctionType.Exp
                        )
                        se = sb.tile([P, G], fdt, tag="se")
                        nc.vector.tensor_reduce(
                            out=se[:],
                            in_=ez[:].rearrange("p (g f) -> p g f", g=G),
                            axis=mybir.AxisListType.X,
                            op=mybir.AluOpType.add,
                        )
                        ls = sb.tile([P, G], fdt, tag="ls")
                        nc.scalar.activation(
                            out=ls[:], in_=se[:], func=mybir.ActivationFunctionType.Ln
                        )
                        out_t = sb.tile([P, G * F], bdt, tag="out")
                        nc.vector.tensor_tensor(
                            out=out_t[:].rearrange("p (g f) -> p g f", g=G),
                            in0=z[:].rearrange("p (g f) -> p g f", g=G),
                            in1=ls[:].unsqueeze(2).broadcast_to([P, G, F]),
                            op=mybir.AluOpType.subtract,
                        )
                        if mlast == P:
                            nc.sync.dma_start(
                                out=y_out[m0 : m0 + G * P, :].rearrange(
                                    "(g p) f -> p g f", g=G
                                ),
                                in_=out_t[:].rearrange("p (g f) -> p g f", g=G),
                            )
                        else:
                            nc.sync.dma_start(
                                out=y_out[m0 : m0 + mlast, :], in_=out_t[:mlast, :]
                            )
                h_prev = h_tab
    nc.compile()
    return nc


def _prep_structure(src, dst):
    """Degree-sorted per-core slot tables.

    Returns (banded idx/mask per core, KT, bands, perm) where
    perm maps new (degree-sorted) global node id -> old global node id.
    """
    deg = np.bincount(dst, minlength=N_NODES)
    assert deg.min() >= 1, "zero in-degree node: reciprocal needs the epsilon path"
    perm = np.empty(N_NODES, np.int64)
    for c in range(N_CORES):
        sl = slice(c * SHARD, (c + 1) * SHARD)
        order_c = np.argsort(-deg[sl], kind="stable")
        perm[sl] = c * SHARD + order_c
    inv_perm = np.empty(N_NODES, np.int64)
    inv_perm[perm] = np.arange(N_NODES)

    ndst = inv_perm[dst]
    nsrc = inv_perm[src]
    order = np.argsort(ndst, kind="stable")
    dsorted = ndst[order]
    ssorted = nsrc[order]
    ndeg = np.bincount(dsorted, minlength=N_NODES)
    starts = np.zeros(N_NODES + 1, np.int64)
    np.cumsum(ndeg, out=starts[1:])
    rank = np.arange(dsorted.shape[0], dtype=np.int64) - starts[dsorted]

    dmat = ndeg.reshape(N_CORES, SHARD)
    KT = []
    for t in range(NT):
        hi = min((t + 1) * P, SHARD)
        KT.append(max(1, int(dmat[:, t * P : hi].max())))

    KMAX = max(KT)
    idx = np.zeros((N_NODES, KMAX), np.int32)
    maskb = np.full((N_NODES, KMAX), -30000.0, np.float32)
    idx[dsorted, rank] = ssorted.astype(np.int32)
    maskb[dsorted, rank] = 0.0

    # width bands over the (non-increasing) KT: 4 bands minimizing padded area
    nb = 4
    INF = 1 << 60
    cost = [[INF] * (nb + 1) for _ in range(NT + 1)]
    prevb = [[-1] * (nb + 1) for _ in range(NT + 1)]
    cost[0][0] = 0
    for t1 in range(1, NT + 1):
        for b in range(1, nb + 1):
            for t0 in range(t1):
                if cost[t0][b - 1] == INF:
                    continue
                w = KT[t0] * (t1 - t0)  # KT non-increasing: band width = KT[t0]
                if cost[t0][b - 1] + w < cost[t1][b]:
                    cost[t1][b] = cost[t0][b - 1] + w
                    prevb[t1][b] = t0
    bands = []
    t1, b = NT, nb
    while t1 > 0:
        t0 = prevb[t1][b]
        bands.append((t0, t1, KT[t0]))
        t1, b = t0, b - 1
    bands.reverse()

    import ml_dtypes

    maskh = maskb.astype(ml_dtypes.bfloat16)
    idx_bc = []   # idx_bc[c][b], mask_bc[c][b]
    mask_bc = []
    for c in range(N_CORES):
        ib_list, mb_list = [], []
        for (t0, t1, Kb) in bands:
            rows = (t1 - t0) * P
            lo = c * SHARD + t0 * P
            hi = min(c * SHARD + t1 * P, (c + 1) * SHARD)
            blk_i = np.zeros((rows, Kb), np.int32)
            blk_m = np.full((rows, Kb), -30000.0, np.float32).astype(ml_dtypes.bfloat16)
            blk_i[: hi - lo] = idx[lo:hi, :Kb]
            blk_m[: hi - lo] = maskh[lo:hi, :Kb]
            ib_list.append(np.ascontiguousarray(blk_i))
            mb_list.append(np.ascontiguousarray(blk_m))
        idx_bc.append(ib_list)
        mask_bc.append(mb_list)
    # batches of consecutive tiles within a band: G*Kb bounded by SBUF budget
    ELEM1 = 2 * _LAYERS[0][1] * _LAYERS[0][2]
    batches = []
    for b, (t0, t1, Kb) in enumerate(bands):
        t = t0
        while t < t1:
            G = 1
            while (
                t + G < t1
                and (G + 1) * Kb * ELEM1 * 2 <= 45056  # 44KB/partition for g2
                and G < 8
                and t + G != NT - 1  # keep the ragged last tile in its own batch
            ):
                G += 1
            if t == NT - 1 or t + G > NT - 1:
                G = min(G, max(1, NT - 1 - t)) if t < NT - 1 else 1
            batches.append((t, G, b))
            t += G
    return idx_bc, mask_bc, KT, bands, batches, perm


def _fold_w(W4, b4, cin, scale_q, F):
    # W4/b4 arrive in q|s|k|v column order; scale applies to the q block
    import ml_dtypes

    kf = 2 if cin + 1 > 128 else 1
    w = np.zeros((128 * kf, 4 * F), np.float32)
    w[:cin] = W4
    w[cin] = b4
    w[:, 0:F] *= scale_q
    return (
        w.astype(ml_dtypes.bfloat16).reshape(kf, 128, 4 * F).transpose(1, 0, 2).copy()
    )


def _get_program(KT, BANDS, BATCHES):
    key = (tuple(KT), tuple(BANDS), tuple(BATCHES))
    if _STATE.get("key") != key:
        nc = _build_program(KT, BANDS, BATCHES)
        import ml_dtypes
        from concourse import bass2jax

        dummy = []
        for _ in range(N_CORES):
            d = {
                "xt1": np.zeros((131, SHARD), ml_dtypes.bfloat16),
            }
            for b, (t0, t1, Kb) in enumerate(BANDS):
                rows = (t1 - t0) * P
                d[f"idx{b}"] = np.zeros((rows, Kb), np.int32)
                d[f"mask{b}"] = np.full(
                    (rows, Kb), -30000.0, np.float32
                ).astype(ml_dtypes.bfloat16)
            for li, (cin, H, D) in enumerate(_LAYERS):
                kf = 2 if cin + 1 > 128 else 1
                d[f"w{li+1}"] = np.zeros((128, kf, 4 * H * D), ml_dtypes.bfloat16)
            dummy.append(d)
        bass2jax.run_bass_via_pjrt(nc, dummy, n_cores=N_CORES)
        _STATE["key"] = key
        _STATE["nc"] = nc
    return _STATE["nc"]


def kernel(**inputs):
    import ml_dtypes
    from concourse.bass_utils import run_bass_kernel_spmd

    x = np.asarray(inputs["x"], np.float32)
    edge_index = np.asarray(inputs["edge_index"])
    src = edge_index[0].astype(np.int64)
    dst = edge_index[1].astype(np.int64)

    idx_bc, mask_bc, KT, bands, batches, perm = _prep_structure(src, dst)
    nc = _get_program(KT, bands, batches)

    ws = []
    for li, (cin, H, D) in enumerate(_LAYERS):
        W4 = np.concatenate(
            [
                np.asarray(inputs[f"W{nm}{li+1}"], np.float32)
                for nm in ["q", "s", "k", "v"]
            ],
            axis=1,
        )
        b4 = np.concatenate(
            [
                np.asarray(inputs[f"b{nm}{li+1}"], np.float32)
                for nm in ["q", "s", "k", "v"]
            ]
        )
        ws.append(_fold_w(W4, b4, cin, 1.0 / np.sqrt(np.float32(D)), H * D))

    xp = x[perm]  # rows in new (degree-sorted) order
    xb = xp.astype(ml_dtypes.bfloat16)
    ones = np.ones((1, SHARD), ml_dtypes.bfloat16)
    in_maps = []
    for c in range(N_CORES):
        sl = slice(c * SHARD, (c + 1) * SHARD)
        xt1 = np.ascontiguousarray(np.concatenate([xb[sl].T, ones], axis=0))
        m = {"xt1": xt1}
        for b in range(len(bands)):
            m[f"idx{b}"] = idx_bc[c][b]
            m[f"mask{b}"] = mask_bc[c][b]
        for li in range(3):
            m[f"w{li+1}"] = ws[li]
        in_maps.append(m)

    import time as _time

    t0 = _time.time()
    res = run_bass_kernel_spmd(nc, in_maps, list(range(N_CORES)))
    dt = int((_time.time() - t0) * 1e9)
    globals()["_DEVICE_WALL_NS"] = globals().get("_DEVICE_WALL_NS", 0) + dt
    globals().setdefault("_LAUNCH_NS", []).append(dt)

    y_perm = np.concatenate(
        [res.results[c]["y"].astype(np.float32) for c in range(N_CORES)], axis=0
    )
    y = np.empty_like(y_perm)
    y[perm] = y_perm  # un-permute rows back to original node order
    return y
